# revision 59
# baseline (speedup 1.0000x reference)
"""Trainium2 Bass kernel for nn_Attention_48799418417201.

Multi-head attention (B=8, S=1024, E=768, H=12, D=64) with LoRA (R=16) on the
QKV projections. Data-parallel over batch: one batch element per NeuronCore,
8 cores.

Layout strategy (per core):
  - LoRA is folded into the projection weights on the host (W_eff = W + B@A),
    the K bias is dropped (softmax-invariant), and the V bias is folded into
    the output-projection bias (attention rows sum to 1).
  - Q/K projections run in fp8e4 with DoubleRow perf mode (256-wide
    contraction per pass, 2x PE throughput).  Weights are scaled by 64 on the
    host so they sit in fp8's normal range; the compensating 1/(64*64) and
    the 1/sqrt(D) scaling ride the exp activation's free `scale` argument.
  - Scores are computed transposed: S^T[j, i] = sum_d K^T[d,j] Q^T[d,i].
    Head pairs pack into one 2-bank PSUM tile per (t, j, i): cols 0-511 =
    head 2t (PE rows 0-63), cols 512-1023 = head 2t+1 (rows 64-127) -- the
    two matmuls hit disjoint PE row groups + PSUM banks and run concurrently.
    One exp covers both heads; the ones-column in V_aug makes the PV matmul
    emit the softmax denominator into PSUM row 64 for free.
  - The projections run n-tile-major with fully resident fp8 activations so
    the first scores/exp fire ~13us into the kernel; score units are paced
    into the projection/V-projection/PV emission streams to keep ScalarE
    (the eventual bottleneck at ~119us of exp work) continuously fed.
  - PV produces O^T [E, S] directly; no on-device transposes anywhere.
  - A warm-up burst of tiny matmuls at t=0 keeps the PE HAM activity monitor
    busy through the DMA lead-in so real matmuls start at 2.4 GHz.
"""

import numpy as np
import ml_dtypes
from contextlib import ExitStack

import concourse.bass as bass
import concourse.bacc as bacc
import concourse.tile as tile
from concourse import mybir
from concourse.bass_utils import run_bass_kernel_spmd

P = 128
S = 1024  # sequence length
E = 768  # embedding
H = 12  # heads
D = 64  # head dim
NT = E // P  # 6 n-tiles (also e-tiles) per 768-wide dim
MC = S // 512  # 2 moving-chunks of 512 along sequence
MS = S // P  # 8 sequence subtiles of 128
JT = S // P  # 8 j-tiles (key blocks)
IC = S // 512  # 2 i-chunks (query blocks of 512)
VW = D + 1  # 65 columns per head in V_aug
WS = 64.0  # fp8 weight scale for q/k projections
EXP_SCALE = float(D) ** -0.5 / (WS * WS)

F16 = mybir.dt.float16
F32 = mybir.dt.float32
F8 = mybir.dt.float8e4
DR = mybir.MatmulPerfMode.DoubleRow


def build_nc():
    nc = bacc.Bacc("TRN2", target_bir_lowering=False, debug=False, num_devices=8)

    xq_d = nc.dram_tensor("xqT", [P, NT, S], F8, kind="ExternalInput")
    xk_d = nc.dram_tensor("xkT", [P, NT, S], F8, kind="ExternalInput")
    xv_d = nc.dram_tensor("xvT", [P, NT, S], F16, kind="ExternalInput")
    wq8_d = nc.dram_tensor("wq8T", [P, NT, NT, P], F8, kind="ExternalInput")
    wk8_d = nc.dram_tensor("wk8T", [P, NT, NT, P], F8, kind="ExternalInput")
    wv_d = nc.dram_tensor("wvT", [P, NT, E], F16, kind="ExternalInput")
    woT_d = nc.dram_tensor("woT", [E, E], F16, kind="ExternalInput")
    bq_d = nc.dram_tensor("bq", [P, NT], F32, kind="ExternalInput")
    ob_d = nc.dram_tensor("ob", [E], F16, kind="ExternalInput")
    out_d = nc.dram_tensor("out", [S, E], F32, kind="ExternalOutput")

    with tile.TileContext(nc) as tc, ExitStack() as perm:
        pp = perm.enter_context(tc.tile_pool(name="perm", bufs=1))

        QT = [pp.tile([P, S], F16, name=f"QT{t}", tag=f"QT{t}") for t in range(NT)]
        KT = [pp.tile([P, S], F16, name=f"KT{t}", tag=f"KT{t}") for t in range(NT)]
        Va = [pp.tile([P, H * VW], F16, name=f"Va{m}", tag=f"Va{m}") for m in range(MS)]
        # per-i-chunk tiles: out-proj m-blocks 0-3 depend only on the i0
        # halves, so they are not fenced behind the i1 normalize chains
        OTu = [[pp.tile([P, 512], F16, name=f"OTu{t}_{i}", tag=f"OTu{t}_{i}")
                for i in range(IC)] for t in range(NT)]
        woT = [pp.tile([P, E], F16, name=f"woT{t}", tag=f"woT{t}") for t in range(NT)]
        bq_sb = pp.tile([P, NT], F32, name="bq_sb", tag="bq_sb")
        zbias = pp.tile([P, 1], F32, name="zbias", tag="zbias")
        ob_sb = pp.tile([P, E], F16, name="ob_sb", tag="ob_sb")
        wu = pp.tile([P, P], F16, name="wu", tag="wu")
        dmy = pp.tile([P, 1], F16, name="dmy", tag="dmy")

        nc.vector.memset(zbias[:], 0.0)
        nc.vector.memset(wu[:], 0.0)
        nc.sync.dma_start(bq_sb[:], bq_d.ap()[:])
        # dummy exp at t=0 absorbs the one-time ~2.7us activation-table load
        # during the DMA lead-in instead of stalling the first real exp
        nc.scalar.activation(
            dmy[:], zbias[:], mybir.ActivationFunctionType.Exp, bias=zbias[:]
        )

        # ---------------- pools ----------------
        # PSUM bank budget (8): ppsum 4 + stp 2x2 = 8.
        ppsum = tc.alloc_tile_pool(name="ppsum", bufs=4, space="PSUM")
        xp = tc.alloc_tile_pool(name="xp", bufs=2)
        stp = tc.alloc_tile_pool(name="stp", bufs=2, space="PSUM")
        ep = tc.alloc_tile_pool(name="ep", bufs=41)
        sgp = tc.alloc_tile_pool(name="sgp", bufs=2)
        zbp = tc.alloc_tile_pool(name="zbp", bufs=3)
        zsp = tc.alloc_tile_pool(name="zsp", bufs=1)
        dpool = tc.alloc_tile_pool(name="dpool", bufs=1, space="DRAM")
        wqk = tc.alloc_tile_pool(name="wqk", bufs=1)
        zdram = dpool.tile([H, S], F32, name="zdram", tag="zdram")

        # HAM warm-up: tiny matmuls with no DMA deps keep the PE busy from
        # t=0 through the input-DMA lead-in so HAM un-throttles early.
        wups = ppsum.tile([16, P], F32, name="wups", tag="acc")
        for _ in range(42):
            nc.tensor.matmul(wups[:], wu[:, :16], wu[:])

        _padn = [0]

        def emit_pad(n):
            # HAM keep-alive filler: runs only when the FIFO is stalled on
            # DMA, preventing a mid-leadin re-throttle to 1.2 GHz.
            _padn[0] += 1
            pt = ppsum.tile([16, 64], F32, name=f"pad{_padn[0]}", tag="acc")
            for _ in range(n):
                nc.tensor.matmul(pt[:], wu[:, :16], wu[:, :64])

        # resident fp8 activations + weights for the q/k projections, fp16
        # weights for the v projection (all released together after v-proj)
        x8 = {
            "q": wqk.tile([P, NT, S], F8, name="xq8", tag="xq8"),
            "k": wqk.tile([P, NT, S], F8, name="xk8", tag="xk8"),
        }
        # weights laid out [p, n-tile, ksub, col]: each n-pair chunk is a
        # contiguous DMA slab, so k-n0's weights land ~10us earlier and the
        # first scores/exp fire at ~12us instead of ~20us
        w8 = {
            "q": wqk.tile([P, NT, NT, P], F8, name="wq8", tag="wq8"),
            "k": wqk.tile([P, NT, NT, P], F8, name="wk8", tag="wk8"),
        }
        wv = wqk.tile([P, NT, E], F16, name="wv", tag="wv")

        # DMA order matches first-use: q weights + m0 x chunks first, so the
        # first projection matmul can start ~6us in; k's m0 path next (first
        # scores ~13us); m1 chunks stream in behind.
        xsrc = {"q": xq_d, "k": xk_d}
        wsrc8 = {"q": wq8_d, "k": wk8_d}

        def emit_x8_dma(name, m):
            msl = slice(m * 512, (m + 1) * 512)
            nc.sync.dma_start(x8[name][:, :, msl], xsrc[name].ap()[:, :, msl])

        def emit_w8_dma(name, ng):
            nsl = slice(2 * ng, 2 * ng + 2)
            nc.sync.dma_start(
                w8[name][:, nsl, :, :], wsrc8[name].ap()[:, nsl, :, :]
            )

        emit_x8_dma("q", 0)
        emit_w8_dma("q", 0)
        emit_x8_dma("k", 0)
        emit_w8_dma("k", 0)
        emit_w8_dma("q", 1)
        emit_w8_dma("k", 1)
        emit_w8_dma("q", 2)
        emit_w8_dma("k", 2)
        emit_x8_dma("q", 1)
        emit_x8_dma("k", 1)

        # ---------------- scores units + pacing ----------------
        # Units become available as their QT/KT slices land; pump() always
        # emits the lowest-(t, i, j) available unit (pv consumption order).
        exps = {}
        savail = []

        def emit_s_unit():
            savail.sort(key=lambda u: (u[0], u[2], u[1]))
            t, j, i = savail.pop(0)
            jsl = slice(j * P, (j + 1) * P)
            isl = slice(i * 512, (i + 1) * 512)
            st = stp.tile([P, 1024], F32, name=f"st{t}_{j}_{i}", tag="st")
            for hh in range(2):
                base = hh * D
                nc.tensor.matmul(
                    st[:, hh * 512 : (hh + 1) * 512],
                    KT[t][base : base + D, jsl],
                    QT[t][base : base + D, isl],
                )
            ex = ep.tile([P, 1024], F16, name=f"ex{t}_{j}_{i}", tag="ex")
            nc.scalar.activation(
                ex[:], st[:], mybir.ActivationFunctionType.Exp,
                bias=zbias[:], scale=EXP_SCALE,
            )
            exps[(t, j, i)] = ex

        def pump(k):
            for _ in range(min(k, len(savail))):
                emit_s_unit()

        # ---------------- q/k projections (fp8 DoubleRow) ----------------
        def emit_proj_qk_nm(name, n, m):
            dest = QT if name == "q" else KT
            nsl = slice(n * P, (n + 1) * P)
            msl = slice(m * 512, (m + 1) * 512)
            acc = ppsum.tile([P, 512], F32, name=f"a{name}{n}_{m}", tag="acc")
            for kk in range(3):
                nc.tensor.matmul(
                    acc[:],
                    w8[name][:, n, 2 * kk : 2 * kk + 2, :],
                    x8[name][:, 2 * kk : 2 * kk + 2, msl],
                    start=(kk == 0), stop=(kk == 2),
                    perf_mode=DR,
                )
            if name == "q":
                nc.vector.tensor_scalar_add(
                    dest[n][:, msl], acc[:], bq_sb[:, n : n + 1]
                )
            else:
                nc.vector.tensor_copy(dest[n][:, msl], acc[:])

        # ---------------- v projection (fp16, x-stationary) ----------------
        def emit_xv_dma(m):
            msl = slice(m * 512, (m + 1) * 512)
            xc = xp.tile([P, NT, 512], F16, name=f"xcv{m}", tag="xc")
            nc.sync.dma_start(xc[:], xv_d.ap()[:, :, msl])
            return xc

        def emit_wv_dma():
            nc.sync.dma_start(wv[:], wv_d.ap()[:])

        def emit_v_setup():
            for g in range(MS):
                va_cols = Va[g].rearrange("p (h c) -> p h c", c=VW)
                nc.vector.memset(va_cols[:, :, D], 1.0)

        def emit_proj_v_g(xc, m, ms_i):
            g = m * 4 + ms_i
            for nch in range(2):
                ncols = 512 if nch == 0 else E - 512
                nh = ncols // D
                nsl = slice(nch * 512, nch * 512 + ncols)
                acc = ppsum.tile([P, 512], F32, name=f"av{g}_{nch}", tag="acc")
                for k in range(NT):
                    nc.tensor.matmul(
                        acc[:, :ncols],
                        xc[:, k, ms_i * P : (ms_i + 1) * P],
                        wv[:, k, nsl],
                        start=(k == 0), stop=(k == NT - 1),
                    )
                h0 = nch * 8
                dst = Va[g].rearrange("p (h c) -> p h c", c=VW)
                src = acc[:, :ncols].rearrange("p (h c) -> p h c", c=D)
                nc.vector.tensor_copy(dst[:, h0 : h0 + nh, 0:D], src[:])

        # ---------------- PV (one head pair, interleaved with pacing) ------
        # PSUM row 64 collects Z (ones column); 1/Z is computed into spare
        # PSUM row 65 so the stage copy carries it out with the O^T rows,
        # then a DRAM-bounce DMA broadcasts it across the 64 head partitions.
        def emit_pv(t, pumps, zb_pre=None):
            # Z sits in PSUM row 64 (ones column); the stage copy carries it
            # to SBUF, a DMA hop moves it to partitions 0/1, DVE computes the
            # reciprocal, and a DRAM-bounce DMA broadcasts it per head.  For
            # the last pair the whole chain is precomputed (emit_z5), so
            # zb_pre skips it.
            ci = 0
            for i in range(IC):
                dq = nc.scalar if (t >= 4 and i == 1) else nc.sync
                isl = slice(i * 512, (i + 1) * 512)
                if zb_pre is None:
                    zb = zbp.tile([P, 512], F32, name=f"zb{t}_{i}", tag="zb")
                    zt = zsp.tile([2, 512], F16, name=f"zt{t}_{i}", tag="zt")
                else:
                    zb = zb_pre[i]
                for hh in range(2):
                    h = 2 * t + hh
                    base = hh * D
                    pv = ppsum.tile([P, 512], F32, name=f"pv{h}_{i}", tag="acc")
                    for j in range(JT):
                        nc.tensor.matmul(
                            pv[0:VW, :],
                            Va[j][:, h * VW : (h + 1) * VW],
                            exps[(t, j, i)][:, hh * 512 : (hh + 1) * 512],
                            start=(j == 0), stop=(j == JT - 1),
                        )
                    stage = sgp.tile([VW, 512], F16, name=f"stg{h}_{i}", tag="stg")
                    nc.vector.tensor_copy(stage[:], pv[0:VW, :])
                    nc.sync.dma_start(OTu[t][i][base : base + D, :], stage[0:D, :])
                    if zb_pre is None:
                        dq.dma_start(zt[hh : hh + 1, :], stage[D : D + 1, :])
                    pump(pumps[ci])
                    ci += 1
                if zb_pre is None:
                    z32 = zsp.tile([2, 512], F32, name=f"z32_{t}_{i}", tag="z32")
                    rz = zsp.tile([2, 512], F32, name=f"rz{t}_{i}", tag="rz")
                    nc.vector.tensor_copy(z32[:], zt[:])
                    nc.vector.reciprocal_approx_fast(rz[:], z32[:])
                    dq.dma_start(zdram[2 * t : 2 * t + 2, isl], rz[:])
                    for hh in range(2):
                        dq.dma_start(
                            zb[hh * D : (hh + 1) * D, :],
                            zdram[2 * t + hh, isl].partition_broadcast(D),
                        )
                nc.vector.tensor_mul(OTu[t][i][:], OTu[t][i][:], zb[:])


        # ---------------- emission sequence ----------------
        # m0 pass: q/k n-tiles over sequence cols 0-511; (t, j<4, i0) score
        # units only need those cols, so exps start ~13us in.
        # all q n-tiles first: their inputs land ~8us in, and the ~5us of
        # q matmuls exactly covers the wait for the k-path DMAs (~15us),
        # keeping the PE busy/warm until scores can start
        for n in range(NT):
            emit_proj_qk_nm("q", n, 0)
        for n in range(NT):
            emit_proj_qk_nm("k", n, 0)
            savail.extend((n, j, 0) for j in range(4))
            pump(2)
        emit_pad(8)
        for n in range(NT):
            emit_proj_qk_nm("q", n, 1)
            emit_proj_qk_nm("k", n, 1)
            savail.extend((n, j, 0) for j in range(4, JT))
            savail.extend((n, j, 1) for j in range(JT))
            if n <= 1:
                emit_pad(8)
            pump(2)
        emit_v_setup()
        emit_wv_dma()
        for m in range(MC):
            xc = emit_xv_dma(m)
            for ms_i in range(4):
                emit_proj_v_g(xc, m, ms_i)
                pump(2 if m == 0 else 3)
        wqk.release()
        for t in range(NT):
            nc.sync.dma_start(woT[t][:], woT_d.ap()[t * P : (t + 1) * P, :])
        nc.sync.dma_start(ob_sb[:], ob_d.ap().partition_broadcast(P))

        pv_pumps = {0: (5, 5, 5, 5), 1: (4, 4, 4, 4), 2: (4, 4, 4, 4),
                    3: (2, 2, 2, 2), 4: (0, 0, 0, 0), 5: (0, 0, 0, 0)}
        for t in range(NT):
            emit_pv(t, pv_pumps[t])
            if t == 3:
                pump(len(savail))
        assert not savail
        # keep the PE's HAM activity monitor warm through the ~10us z-chain
        # wait between pv5 and the output projection, so the out matmuls
        # start at 2.4 GHz instead of re-throttled 1.2 GHz
        emit_pad(64)

        # ---------------- Phase O: output projection ----------------
        # Runs out of the still-live stp (PSUM accs) and ep (fp32 staging)
        # pools -- no pool-release fence between pv5 and the first out matmul.
        for m in range(MS):
            mi, mo = m // 4, m % 4
            acc = stp.tile([P, S], F32, name=f"oacc{m}", tag="st")
            # e=5 sits in its own accumulation group so the wait on the last
            # head pair's normalize chain lands on that matmul alone, not on
            # the group head (m0 e0 can start the moment pv5's matmuls end)
            for e in range(NT):
                for nch in range(2):
                    ncols = 512 if nch == 0 else E - 512
                    nsl = slice(nch * 512, nch * 512 + ncols)
                    nc.tensor.matmul(
                        acc[:, nsl],
                        OTu[e][mi][:, mo * P : (mo + 1) * P],
                        woT[e][:, nsl],
                        start=(e == 0),
                        stop=(e >= NT - 2),
                        skip_group_check=True,
                    )
            fin0 = ep.tile([P, 512], F32, name=f"fin{m}a", tag="ex")
            fin1 = ep.tile([P, E - 512], F32, name=f"fin{m}b", tag="ex")
            nc.vector.tensor_add(fin0[:], acc[:, 0:512], ob_sb[:, 0:512])
            nc.vector.tensor_add(fin1[:], acc[:, 512:E], ob_sb[:, 512:E])
            nc.sync.dma_start(out_d.ap()[m * P : (m + 1) * P, 0:512], fin0[:])
            nc.sync.dma_start(out_d.ap()[m * P : (m + 1) * P, 512:E], fin1[:])

        dpool.release()
        zsp.release()
        zbp.release()
        sgp.release()
        ep.release()
        stp.release()
        xp.release()
        ppsum.release()

    nc.compile()
    return nc


def _prep_inputs(q, k, v, in_proj_weight, in_proj_bias, out_w, out_b, lora_a, lora_b):
    q = np.asarray(q, np.float32)
    k = np.asarray(k, np.float32)
    v = np.asarray(v, np.float32)
    in_proj_weight = np.asarray(in_proj_weight, np.float32)
    in_proj_bias = np.asarray(in_proj_bias, np.float32)
    out_w = np.asarray(out_w, np.float32)
    out_b = np.asarray(out_b, np.float32)
    lora_a = np.asarray(lora_a, np.float32)
    lora_b = np.asarray(lora_b, np.float32)

    # Fold LoRA into the projection weights; drop the K bias
    # (softmax-invariant); fold the V bias into the output-projection bias
    # (attention rows sum to 1).  Q/K weights scaled by WS for fp8; the
    # compensation (and 1/sqrt(D)) is applied by the exp activation's scale.
    w_eff = in_proj_weight + lora_b @ lora_a  # [3E, E]
    wT = w_eff.T  # [E, 3E]
    w8 = np.clip(WS * wT[:, : 2 * E], -240, 240).astype(ml_dtypes.float8_e4m3)
    bq = (WS * in_proj_bias[:E]).reshape(NT, P).T  # [P, NT]
    bv = in_proj_bias[2 * E :]
    ob_eff = out_b + out_w @ bv

    f8c = lambda a: np.clip(a, -240, 240).astype(ml_dtypes.float8_e4m3)

    def pmaj(a):  # [E, X] -> [P, NT, X] partition-major (contiguous DMA lines)
        return np.ascontiguousarray(a.reshape(NT, P, -1).transpose(1, 0, 2))

    def wmaj(a):  # [E_in, E_out] -> [P, n, ksub, 128] with contiguous n-chunks
        return np.ascontiguousarray(
            a.reshape(NT, P, NT, P).transpose(1, 2, 0, 3)
        )

    shared = {
        "wq8T": wmaj(w8[:, :E]),
        "wk8T": wmaj(w8[:, E : 2 * E]),
        "wvT": pmaj(wT[:, 2 * E :].astype(np.float16)),
        "woT": np.ascontiguousarray(out_w.T, np.float16),
        "bq": np.ascontiguousarray(bq, np.float32),
        "ob": np.ascontiguousarray(ob_eff, np.float16),
    }
    in_maps = []
    for b in range(8):
        m = dict(shared)
        m["xqT"] = pmaj(f8c(q[b].T))
        m["xkT"] = pmaj(f8c(k[b].T))
        m["xvT"] = pmaj(v[b].T.astype(np.float16))
        in_maps.append(m)
    return in_maps


_NC_CACHE = {}


def run(inputs, trace=False, **spmd_kwargs):
    if "nc" not in _NC_CACHE:
        _NC_CACHE["nc"] = build_nc()
    nc = _NC_CACHE["nc"]
    in_maps = _prep_inputs(
        inputs["q"],
        inputs["k"],
        inputs["v"],
        inputs["in_proj_weight"],
        inputs["in_proj_bias"],
        inputs["out_w"],
        inputs["out_b"],
        inputs["lora_a"],
        inputs["lora_b"],
    )
    res = run_bass_kernel_spmd(
        nc, in_maps, core_ids=list(range(8)), trace=trace, **spmd_kwargs
    )
    out = np.stack([res.results[b]["out"] for b in range(8)]).astype(np.float32)
    return out, res


def kernel(
    q,
    k,
    v,
    in_proj_weight,
    in_proj_bias,
    out_w,
    out_b,
    lora_a,
    lora_b,
    num_heads=12,
    **_unused,
):
    assert int(num_heads) == H
    out, _ = run(
        {
            "q": q,
            "k": k,
            "v": v,
            "in_proj_weight": in_proj_weight,
            "in_proj_bias": in_proj_bias,
            "out_w": out_w,
            "out_b": out_b,
            "lora_a": lora_a,
            "lora_b": lora_b,
        }
    )
    return out


# revision 60
# speedup vs baseline: 1.0026x; 1.0026x over previous
"""Trainium2 Bass kernel for nn_Attention_48799418417201.

Multi-head attention (B=8, S=1024, E=768, H=12, D=64) with LoRA (R=16) on the
QKV projections. Data-parallel over batch: one batch element per NeuronCore,
8 cores.

Layout strategy (per core):
  - LoRA is folded into the projection weights on the host (W_eff = W + B@A),
    the K bias is dropped (softmax-invariant), and the V bias is folded into
    the output-projection bias (attention rows sum to 1).
  - Q/K projections run in fp8e4 with DoubleRow perf mode (256-wide
    contraction per pass, 2x PE throughput).  Weights are scaled by 64 on the
    host so they sit in fp8's normal range; the compensating 1/(64*64) and
    the 1/sqrt(D) scaling ride the exp activation's free `scale` argument.
  - Scores are computed transposed: S^T[j, i] = sum_d K^T[d,j] Q^T[d,i].
    Head pairs pack into one 2-bank PSUM tile per (t, j, i): cols 0-511 =
    head 2t (PE rows 0-63), cols 512-1023 = head 2t+1 (rows 64-127) -- the
    two matmuls hit disjoint PE row groups + PSUM banks and run concurrently.
    One exp covers both heads; the ones-column in V_aug makes the PV matmul
    emit the softmax denominator into PSUM row 64 for free.
  - The projections run n-tile-major with fully resident fp8 activations so
    the first scores/exp fire ~13us into the kernel; score units are paced
    into the projection/V-projection/PV emission streams to keep ScalarE
    (the eventual bottleneck at ~119us of exp work) continuously fed.
  - PV produces O^T [E, S] directly; no on-device transposes anywhere.
  - A warm-up burst of tiny matmuls at t=0 keeps the PE HAM activity monitor
    busy through the DMA lead-in so real matmuls start at 2.4 GHz.
"""

import numpy as np
import ml_dtypes
from contextlib import ExitStack

import concourse.bass as bass
import concourse.bacc as bacc
import concourse.tile as tile
from concourse import mybir
from concourse.bass_utils import run_bass_kernel_spmd

P = 128
S = 1024  # sequence length
E = 768  # embedding
H = 12  # heads
D = 64  # head dim
NT = E // P  # 6 n-tiles (also e-tiles) per 768-wide dim
MC = S // 512  # 2 moving-chunks of 512 along sequence
MS = S // P  # 8 sequence subtiles of 128
JT = S // P  # 8 j-tiles (key blocks)
IC = S // 512  # 2 i-chunks (query blocks of 512)
VW = D + 1  # 65 columns per head in V_aug
WS = 64.0  # fp8 weight scale for q/k projections
EXP_SCALE = float(D) ** -0.5 / (WS * WS)

F16 = mybir.dt.float16
F32 = mybir.dt.float32
F8 = mybir.dt.float8e4
DR = mybir.MatmulPerfMode.DoubleRow


def build_nc():
    nc = bacc.Bacc("TRN2", target_bir_lowering=False, debug=False, num_devices=8)

    xq_d = nc.dram_tensor("xqT", [P, NT, S], F8, kind="ExternalInput")
    xk_d = nc.dram_tensor("xkT", [P, NT, S], F8, kind="ExternalInput")
    xv_d = nc.dram_tensor("xvT", [P, NT, S], F16, kind="ExternalInput")
    wq8_d = nc.dram_tensor("wq8T", [P, NT, NT, P], F8, kind="ExternalInput")
    wk8_d = nc.dram_tensor("wk8T", [P, NT, NT, P], F8, kind="ExternalInput")
    wv_d = nc.dram_tensor("wvT", [P, NT, E], F16, kind="ExternalInput")
    woT_d = nc.dram_tensor("woT", [E, E], F16, kind="ExternalInput")
    bq_d = nc.dram_tensor("bq", [P, NT], F32, kind="ExternalInput")
    ob_d = nc.dram_tensor("ob", [E], F16, kind="ExternalInput")
    out_d = nc.dram_tensor("out", [S, E], F32, kind="ExternalOutput")

    with tile.TileContext(nc) as tc, ExitStack() as perm:
        pp = perm.enter_context(tc.tile_pool(name="perm", bufs=1))

        QT = [pp.tile([P, S], F16, name=f"QT{t}", tag=f"QT{t}") for t in range(NT)]
        KT = [pp.tile([P, S], F16, name=f"KT{t}", tag=f"KT{t}") for t in range(NT)]
        Va = [pp.tile([P, H * VW], F16, name=f"Va{m}", tag=f"Va{m}") for m in range(MS)]
        # per-i-chunk tiles: out-proj m-blocks 0-3 depend only on the i0
        # halves, so they are not fenced behind the i1 normalize chains
        OTu = [[pp.tile([P, 512], F16, name=f"OTu{t}_{i}", tag=f"OTu{t}_{i}")
                for i in range(IC)] for t in range(NT)]
        woT = [pp.tile([P, E], F16, name=f"woT{t}", tag=f"woT{t}") for t in range(NT)]
        bq_sb = pp.tile([P, NT], F32, name="bq_sb", tag="bq_sb")
        zbias = pp.tile([P, 1], F32, name="zbias", tag="zbias")
        ob_sb = pp.tile([P, E], F16, name="ob_sb", tag="ob_sb")
        wu = pp.tile([P, P], F16, name="wu", tag="wu")
        dmy = pp.tile([P, 1], F16, name="dmy", tag="dmy")

        nc.vector.memset(zbias[:], 0.0)
        nc.vector.memset(wu[:], 0.0)
        nc.sync.dma_start(bq_sb[:], bq_d.ap()[:])
        # dummy exp at t=0 absorbs the one-time ~2.7us activation-table load
        # during the DMA lead-in instead of stalling the first real exp
        nc.scalar.activation(
            dmy[:], zbias[:], mybir.ActivationFunctionType.Exp, bias=zbias[:]
        )

        # ---------------- pools ----------------
        # PSUM bank budget (8): ppsum 4 + stp 2x2 = 8.
        ppsum = tc.alloc_tile_pool(name="ppsum", bufs=4, space="PSUM")
        xp = tc.alloc_tile_pool(name="xp", bufs=2)
        stp = tc.alloc_tile_pool(name="stp", bufs=2, space="PSUM")
        ep = tc.alloc_tile_pool(name="ep", bufs=41)
        sgp = tc.alloc_tile_pool(name="sgp", bufs=2)
        zbp = tc.alloc_tile_pool(name="zbp", bufs=3)
        zsp = tc.alloc_tile_pool(name="zsp", bufs=1)
        dpool = tc.alloc_tile_pool(name="dpool", bufs=1, space="DRAM")
        wqk = tc.alloc_tile_pool(name="wqk", bufs=1)
        zdram = dpool.tile([H, S], F32, name="zdram", tag="zdram")

        # HAM warm-up: tiny matmuls with no DMA deps keep the PE busy from
        # t=0 through the input-DMA lead-in so HAM un-throttles early.
        wups = ppsum.tile([16, P], F32, name="wups", tag="acc")
        for _ in range(20):
            nc.tensor.matmul(wups[:], wu[:, :16], wu[:])

        _padn = [0]

        def emit_pad(n):
            # HAM keep-alive filler: runs only when the FIFO is stalled on
            # DMA, preventing a mid-leadin re-throttle to 1.2 GHz.
            _padn[0] += 1
            pt = ppsum.tile([16, 64], F32, name=f"pad{_padn[0]}", tag="acc")
            for _ in range(n):
                nc.tensor.matmul(pt[:], wu[:, :16], wu[:, :64])

        # resident fp8 activations + weights for the q/k projections, fp16
        # weights for the v projection (all released together after v-proj)
        x8 = {
            "q": wqk.tile([P, NT, S], F8, name="xq8", tag="xq8"),
            "k": wqk.tile([P, NT, S], F8, name="xk8", tag="xk8"),
        }
        # weights laid out [p, n-tile, ksub, col]: each n-pair chunk is a
        # contiguous DMA slab, so k-n0's weights land ~10us earlier and the
        # first scores/exp fire at ~12us instead of ~20us
        w8 = {
            "q": wqk.tile([P, NT, NT, P], F8, name="wq8", tag="wq8"),
            "k": wqk.tile([P, NT, NT, P], F8, name="wk8", tag="wk8"),
        }
        wv = wqk.tile([P, NT, E], F16, name="wv", tag="wv")

        # DMA order matches first-use: q weights + m0 x chunks first, so the
        # first projection matmul can start ~6us in; k's m0 path next (first
        # scores ~13us); m1 chunks stream in behind.
        xsrc = {"q": xq_d, "k": xk_d}
        wsrc8 = {"q": wq8_d, "k": wk8_d}

        def emit_x8_dma(name, m):
            msl = slice(m * 512, (m + 1) * 512)
            nc.sync.dma_start(x8[name][:, :, msl], xsrc[name].ap()[:, :, msl])

        def emit_w8_dma(name, ng):
            nsl = slice(2 * ng, 2 * ng + 2)
            nc.sync.dma_start(
                w8[name][:, nsl, :, :], wsrc8[name].ap()[:, nsl, :, :]
            )

        emit_x8_dma("q", 0)
        emit_w8_dma("q", 0)
        emit_x8_dma("k", 0)
        emit_w8_dma("k", 0)
        emit_w8_dma("q", 1)
        emit_w8_dma("k", 1)
        emit_w8_dma("q", 2)
        emit_w8_dma("k", 2)
        emit_x8_dma("q", 1)
        emit_x8_dma("k", 1)

        # ---------------- scores units + pacing ----------------
        # Units become available as their QT/KT slices land; pump() always
        # emits the lowest-(t, i, j) available unit (pv consumption order).
        exps = {}
        savail = []

        def emit_s_unit():
            savail.sort(key=lambda u: (u[0], u[2], u[1]))
            t, j, i = savail.pop(0)
            jsl = slice(j * P, (j + 1) * P)
            isl = slice(i * 512, (i + 1) * 512)
            st = stp.tile([P, 1024], F32, name=f"st{t}_{j}_{i}", tag="st")
            for hh in range(2):
                base = hh * D
                nc.tensor.matmul(
                    st[:, hh * 512 : (hh + 1) * 512],
                    KT[t][base : base + D, jsl],
                    QT[t][base : base + D, isl],
                )
            ex = ep.tile([P, 1024], F16, name=f"ex{t}_{j}_{i}", tag="ex")
            nc.scalar.activation(
                ex[:], st[:], mybir.ActivationFunctionType.Exp,
                bias=zbias[:], scale=EXP_SCALE,
            )
            exps[(t, j, i)] = ex

        def pump(k):
            for _ in range(min(k, len(savail))):
                emit_s_unit()

        # ---------------- q/k projections (fp8 DoubleRow) ----------------
        def emit_proj_qk_nm(name, n, m):
            dest = QT if name == "q" else KT
            nsl = slice(n * P, (n + 1) * P)
            msl = slice(m * 512, (m + 1) * 512)
            acc = ppsum.tile([P, 512], F32, name=f"a{name}{n}_{m}", tag="acc")
            for kk in range(3):
                nc.tensor.matmul(
                    acc[:],
                    w8[name][:, n, 2 * kk : 2 * kk + 2, :],
                    x8[name][:, 2 * kk : 2 * kk + 2, msl],
                    start=(kk == 0), stop=(kk == 2),
                    perf_mode=DR,
                )
            if name == "q":
                nc.vector.tensor_scalar_add(
                    dest[n][:, msl], acc[:], bq_sb[:, n : n + 1]
                )
            else:
                nc.vector.tensor_copy(dest[n][:, msl], acc[:])

        # ---------------- v projection (fp16, x-stationary) ----------------
        def emit_xv_dma(m):
            msl = slice(m * 512, (m + 1) * 512)
            xc = xp.tile([P, NT, 512], F16, name=f"xcv{m}", tag="xc")
            nc.sync.dma_start(xc[:], xv_d.ap()[:, :, msl])
            return xc

        def emit_wv_dma():
            nc.sync.dma_start(wv[:], wv_d.ap()[:])

        def emit_v_setup():
            for g in range(MS):
                va_cols = Va[g].rearrange("p (h c) -> p h c", c=VW)
                nc.vector.memset(va_cols[:, :, D], 1.0)

        def emit_proj_v_g(xc, m, ms_i):
            g = m * 4 + ms_i
            for nch in range(2):
                ncols = 512 if nch == 0 else E - 512
                nh = ncols // D
                nsl = slice(nch * 512, nch * 512 + ncols)
                acc = ppsum.tile([P, 512], F32, name=f"av{g}_{nch}", tag="acc")
                for k in range(NT):
                    nc.tensor.matmul(
                        acc[:, :ncols],
                        xc[:, k, ms_i * P : (ms_i + 1) * P],
                        wv[:, k, nsl],
                        start=(k == 0), stop=(k == NT - 1),
                    )
                h0 = nch * 8
                dst = Va[g].rearrange("p (h c) -> p h c", c=VW)
                src = acc[:, :ncols].rearrange("p (h c) -> p h c", c=D)
                nc.vector.tensor_copy(dst[:, h0 : h0 + nh, 0:D], src[:])

        # ---------------- PV (one head pair, interleaved with pacing) ------
        # PSUM row 64 collects Z (ones column); 1/Z is computed into spare
        # PSUM row 65 so the stage copy carries it out with the O^T rows,
        # then a DRAM-bounce DMA broadcasts it across the 64 head partitions.
        def emit_pv(t, pumps, zb_pre=None):
            # Z sits in PSUM row 64 (ones column); the stage copy carries it
            # to SBUF, a DMA hop moves it to partitions 0/1, DVE computes the
            # reciprocal, and a DRAM-bounce DMA broadcasts it per head.  For
            # the last pair the whole chain is precomputed (emit_z5), so
            # zb_pre skips it.
            ci = 0
            for i in range(IC):
                dq = nc.scalar if (t >= 4 and i == 1) else nc.sync
                isl = slice(i * 512, (i + 1) * 512)
                if zb_pre is None:
                    zb = zbp.tile([P, 512], F32, name=f"zb{t}_{i}", tag="zb")
                    zt = zsp.tile([2, 512], F16, name=f"zt{t}_{i}", tag="zt")
                else:
                    zb = zb_pre[i]
                for hh in range(2):
                    h = 2 * t + hh
                    base = hh * D
                    pv = ppsum.tile([P, 512], F32, name=f"pv{h}_{i}", tag="acc")
                    for j in range(JT):
                        nc.tensor.matmul(
                            pv[0:VW, :],
                            Va[j][:, h * VW : (h + 1) * VW],
                            exps[(t, j, i)][:, hh * 512 : (hh + 1) * 512],
                            start=(j == 0), stop=(j == JT - 1),
                        )
                    stage = sgp.tile([VW, 512], F16, name=f"stg{h}_{i}", tag="stg")
                    nc.vector.tensor_copy(stage[:], pv[0:VW, :])
                    nc.sync.dma_start(OTu[t][i][base : base + D, :], stage[0:D, :])
                    if zb_pre is None:
                        dq.dma_start(zt[hh : hh + 1, :], stage[D : D + 1, :])
                    pump(pumps[ci])
                    ci += 1
                if zb_pre is None:
                    z32 = zsp.tile([2, 512], F32, name=f"z32_{t}_{i}", tag="z32")
                    rz = zsp.tile([2, 512], F32, name=f"rz{t}_{i}", tag="rz")
                    nc.vector.tensor_copy(z32[:], zt[:])
                    nc.vector.reciprocal_approx_fast(rz[:], z32[:])
                    dq.dma_start(zdram[2 * t : 2 * t + 2, isl], rz[:])
                    for hh in range(2):
                        dq.dma_start(
                            zb[hh * D : (hh + 1) * D, :],
                            zdram[2 * t + hh, isl].partition_broadcast(D),
                        )
                nc.vector.tensor_mul(OTu[t][i][:], OTu[t][i][:], zb[:])


        # ---------------- emission sequence ----------------
        # m0 pass: q/k n-tiles over sequence cols 0-511; (t, j<4, i0) score
        # units only need those cols, so exps start ~13us in.
        # all q n-tiles first: their inputs land ~8us in, and the ~5us of
        # q matmuls exactly covers the wait for the k-path DMAs (~15us),
        # keeping the PE busy/warm until scores can start
        for n in range(NT):
            emit_proj_qk_nm("q", n, 0)
        for n in range(NT):
            emit_proj_qk_nm("k", n, 0)
            savail.extend((n, j, 0) for j in range(4))
            pump(2)
        emit_pad(8)
        for n in range(NT):
            emit_proj_qk_nm("q", n, 1)
            emit_proj_qk_nm("k", n, 1)
            savail.extend((n, j, 0) for j in range(4, JT))
            savail.extend((n, j, 1) for j in range(JT))
            if n <= 1:
                emit_pad(8)
            pump(2)
        emit_v_setup()
        emit_wv_dma()
        for m in range(MC):
            xc = emit_xv_dma(m)
            for ms_i in range(4):
                emit_proj_v_g(xc, m, ms_i)
                pump(2 if m == 0 else 3)
        wqk.release()
        for t in range(NT):
            nc.sync.dma_start(woT[t][:], woT_d.ap()[t * P : (t + 1) * P, :])
        nc.sync.dma_start(ob_sb[:], ob_d.ap().partition_broadcast(P))

        pv_pumps = {0: (5, 5, 5, 5), 1: (4, 4, 4, 4), 2: (4, 4, 4, 4),
                    3: (2, 2, 2, 2), 4: (0, 0, 0, 0), 5: (0, 0, 0, 0)}
        for t in range(NT):
            emit_pv(t, pv_pumps[t])
            if t == 3:
                pump(len(savail))
        assert not savail
        # keep the PE's HAM activity monitor warm through the ~10us z-chain
        # wait between pv5 and the output projection, so the out matmuls
        # start at 2.4 GHz instead of re-throttled 1.2 GHz
        emit_pad(64)

        # ---------------- Phase O: output projection ----------------
        # Runs out of the still-live stp (PSUM accs) and ep (fp32 staging)
        # pools -- no pool-release fence between pv5 and the first out matmul.
        for m in range(MS):
            mi, mo = m // 4, m % 4
            acc = stp.tile([P, S], F32, name=f"oacc{m}", tag="st")
            # e=5 sits in its own accumulation group so the wait on the last
            # head pair's normalize chain lands on that matmul alone, not on
            # the group head (m0 e0 can start the moment pv5's matmuls end)
            for e in range(NT):
                for nch in range(2):
                    ncols = 512 if nch == 0 else E - 512
                    nsl = slice(nch * 512, nch * 512 + ncols)
                    nc.tensor.matmul(
                        acc[:, nsl],
                        OTu[e][mi][:, mo * P : (mo + 1) * P],
                        woT[e][:, nsl],
                        start=(e == 0),
                        stop=(e >= NT - 2),
                        skip_group_check=True,
                    )
            fin0 = ep.tile([P, 512], F32, name=f"fin{m}a", tag="ex")
            fin1 = ep.tile([P, E - 512], F32, name=f"fin{m}b", tag="ex")
            nc.vector.tensor_add(fin0[:], acc[:, 0:512], ob_sb[:, 0:512])
            nc.vector.tensor_add(fin1[:], acc[:, 512:E], ob_sb[:, 512:E])
            nc.sync.dma_start(out_d.ap()[m * P : (m + 1) * P, 0:512], fin0[:])
            nc.sync.dma_start(out_d.ap()[m * P : (m + 1) * P, 512:E], fin1[:])

        dpool.release()
        zsp.release()
        zbp.release()
        sgp.release()
        ep.release()
        stp.release()
        xp.release()
        ppsum.release()

    nc.compile()
    return nc


def _prep_inputs(q, k, v, in_proj_weight, in_proj_bias, out_w, out_b, lora_a, lora_b):
    q = np.asarray(q, np.float32)
    k = np.asarray(k, np.float32)
    v = np.asarray(v, np.float32)
    in_proj_weight = np.asarray(in_proj_weight, np.float32)
    in_proj_bias = np.asarray(in_proj_bias, np.float32)
    out_w = np.asarray(out_w, np.float32)
    out_b = np.asarray(out_b, np.float32)
    lora_a = np.asarray(lora_a, np.float32)
    lora_b = np.asarray(lora_b, np.float32)

    # Fold LoRA into the projection weights; drop the K bias
    # (softmax-invariant); fold the V bias into the output-projection bias
    # (attention rows sum to 1).  Q/K weights scaled by WS for fp8; the
    # compensation (and 1/sqrt(D)) is applied by the exp activation's scale.
    w_eff = in_proj_weight + lora_b @ lora_a  # [3E, E]
    wT = w_eff.T  # [E, 3E]
    w8 = np.clip(WS * wT[:, : 2 * E], -240, 240).astype(ml_dtypes.float8_e4m3)
    bq = (WS * in_proj_bias[:E]).reshape(NT, P).T  # [P, NT]
    bv = in_proj_bias[2 * E :]
    ob_eff = out_b + out_w @ bv

    f8c = lambda a: np.clip(a, -240, 240).astype(ml_dtypes.float8_e4m3)

    def pmaj(a):  # [E, X] -> [P, NT, X] partition-major (contiguous DMA lines)
        return np.ascontiguousarray(a.reshape(NT, P, -1).transpose(1, 0, 2))

    def wmaj(a):  # [E_in, E_out] -> [P, n, ksub, 128] with contiguous n-chunks
        return np.ascontiguousarray(
            a.reshape(NT, P, NT, P).transpose(1, 2, 0, 3)
        )

    shared = {
        "wq8T": wmaj(w8[:, :E]),
        "wk8T": wmaj(w8[:, E : 2 * E]),
        "wvT": pmaj(wT[:, 2 * E :].astype(np.float16)),
        "woT": np.ascontiguousarray(out_w.T, np.float16),
        "bq": np.ascontiguousarray(bq, np.float32),
        "ob": np.ascontiguousarray(ob_eff, np.float16),
    }
    in_maps = []
    for b in range(8):
        m = dict(shared)
        m["xqT"] = pmaj(f8c(q[b].T))
        m["xkT"] = pmaj(f8c(k[b].T))
        m["xvT"] = pmaj(v[b].T.astype(np.float16))
        in_maps.append(m)
    return in_maps


_NC_CACHE = {}


def run(inputs, trace=False, **spmd_kwargs):
    if "nc" not in _NC_CACHE:
        _NC_CACHE["nc"] = build_nc()
    nc = _NC_CACHE["nc"]
    in_maps = _prep_inputs(
        inputs["q"],
        inputs["k"],
        inputs["v"],
        inputs["in_proj_weight"],
        inputs["in_proj_bias"],
        inputs["out_w"],
        inputs["out_b"],
        inputs["lora_a"],
        inputs["lora_b"],
    )
    res = run_bass_kernel_spmd(
        nc, in_maps, core_ids=list(range(8)), trace=trace, **spmd_kwargs
    )
    out = np.stack([res.results[b]["out"] for b in range(8)]).astype(np.float32)
    return out, res


def kernel(
    q,
    k,
    v,
    in_proj_weight,
    in_proj_bias,
    out_w,
    out_b,
    lora_a,
    lora_b,
    num_heads=12,
    **_unused,
):
    assert int(num_heads) == H
    out, _ = run(
        {
            "q": q,
            "k": k,
            "v": v,
            "in_proj_weight": in_proj_weight,
            "in_proj_bias": in_proj_bias,
            "out_w": out_w,
            "out_b": out_b,
            "lora_a": lora_a,
            "lora_b": lora_b,
        }
    )
    return out


# revision 61
# speedup vs baseline: 1.0124x; 1.0098x over previous
"""Trainium2 Bass kernel for nn_Attention_48799418417201.

Multi-head attention (B=8, S=1024, E=768, H=12, D=64) with LoRA (R=16) on the
QKV projections. Data-parallel over batch: one batch element per NeuronCore,
8 cores.

Layout strategy (per core):
  - LoRA is folded into the projection weights on the host (W_eff = W + B@A),
    the K bias is dropped (softmax-invariant), and the V bias is folded into
    the output-projection bias (attention rows sum to 1).
  - Q/K projections run in fp8e4 with DoubleRow perf mode (256-wide
    contraction per pass, 2x PE throughput).  Weights are scaled by 64 on the
    host so they sit in fp8's normal range; the compensating 1/(64*64) and
    the 1/sqrt(D) scaling ride the exp activation's free `scale` argument.
  - Scores are computed transposed: S^T[j, i] = sum_d K^T[d,j] Q^T[d,i].
    Head pairs pack into one 2-bank PSUM tile per (t, j, i): cols 0-511 =
    head 2t (PE rows 0-63), cols 512-1023 = head 2t+1 (rows 64-127) -- the
    two matmuls hit disjoint PE row groups + PSUM banks and run concurrently.
    One exp covers both heads; the ones-column in V_aug makes the PV matmul
    emit the softmax denominator into PSUM row 64 for free.
  - The projections run n-tile-major with fully resident fp8 activations so
    the first scores/exp fire ~13us into the kernel; score units are paced
    into the projection/V-projection/PV emission streams to keep ScalarE
    (the eventual bottleneck at ~119us of exp work) continuously fed.
  - PV produces O^T [E, S] directly; no on-device transposes anywhere.
  - A warm-up burst of tiny matmuls at t=0 keeps the PE HAM activity monitor
    busy through the DMA lead-in so real matmuls start at 2.4 GHz.
"""

import numpy as np
import ml_dtypes
from contextlib import ExitStack

import concourse.bass as bass
import concourse.bacc as bacc
import concourse.tile as tile
from concourse import mybir
from concourse.bass_utils import run_bass_kernel_spmd

P = 128
S = 1024  # sequence length
E = 768  # embedding
H = 12  # heads
D = 64  # head dim
NT = E // P  # 6 n-tiles (also e-tiles) per 768-wide dim
MC = S // 512  # 2 moving-chunks of 512 along sequence
MS = S // P  # 8 sequence subtiles of 128
JT = S // P  # 8 j-tiles (key blocks)
IC = S // 512  # 2 i-chunks (query blocks of 512)
VW = D + 1  # 65 columns per head in V_aug
WS = 64.0  # fp8 weight scale for q/k projections
EXP_SCALE = float(D) ** -0.5 / (WS * WS)

F16 = mybir.dt.float16
F32 = mybir.dt.float32
F8 = mybir.dt.float8e4
DR = mybir.MatmulPerfMode.DoubleRow


def build_nc():
    nc = bacc.Bacc("TRN2", target_bir_lowering=False, debug=False, num_devices=8)

    xq_d = nc.dram_tensor("xqT", [P, NT, S], F8, kind="ExternalInput")
    xk_d = nc.dram_tensor("xkT", [P, NT, S], F8, kind="ExternalInput")
    xv_d = nc.dram_tensor("xvT", [P, NT, S], F16, kind="ExternalInput")
    wq8_d = nc.dram_tensor("wq8T", [P, NT, NT, P], F8, kind="ExternalInput")
    wk8_d = nc.dram_tensor("wk8T", [P, NT, NT, P], F8, kind="ExternalInput")
    wv_d = nc.dram_tensor("wvT", [P, NT, E], F16, kind="ExternalInput")
    woT_d = nc.dram_tensor("woT", [E, E], F16, kind="ExternalInput")
    bq_d = nc.dram_tensor("bq", [P, NT], F32, kind="ExternalInput")
    ob_d = nc.dram_tensor("ob", [E], F16, kind="ExternalInput")
    out_d = nc.dram_tensor("out", [S, E], F16, kind="ExternalOutput")

    with tile.TileContext(nc) as tc, ExitStack() as perm:
        pp = perm.enter_context(tc.tile_pool(name="perm", bufs=1))

        QT = [pp.tile([P, S], F16, name=f"QT{t}", tag=f"QT{t}") for t in range(NT)]
        KT = [pp.tile([P, S], F16, name=f"KT{t}", tag=f"KT{t}") for t in range(NT)]
        Va = [pp.tile([P, H * VW], F16, name=f"Va{m}", tag=f"Va{m}") for m in range(MS)]
        # per-i-chunk tiles: out-proj m-blocks 0-3 depend only on the i0
        # halves, so they are not fenced behind the i1 normalize chains
        OTu = [[pp.tile([P, 512], F16, name=f"OTu{t}_{i}", tag=f"OTu{t}_{i}")
                for i in range(IC)] for t in range(NT)]
        woT = [pp.tile([P, E], F16, name=f"woT{t}", tag=f"woT{t}") for t in range(NT)]
        bq_sb = pp.tile([P, NT], F32, name="bq_sb", tag="bq_sb")
        zbias = pp.tile([P, 1], F32, name="zbias", tag="zbias")
        ob_sb = pp.tile([P, E], F16, name="ob_sb", tag="ob_sb")
        wu = pp.tile([P, P], F16, name="wu", tag="wu")
        dmy = pp.tile([P, 1], F16, name="dmy", tag="dmy")

        nc.vector.memset(zbias[:], 0.0)
        nc.vector.memset(wu[:], 0.0)
        nc.sync.dma_start(bq_sb[:], bq_d.ap()[:])
        # dummy exp at t=0 absorbs the one-time ~2.7us activation-table load
        # during the DMA lead-in instead of stalling the first real exp
        nc.scalar.activation(
            dmy[:], zbias[:], mybir.ActivationFunctionType.Exp, bias=zbias[:]
        )

        # ---------------- pools ----------------
        # PSUM bank budget (8): ppsum 4 + stp 2x2 = 8.
        ppsum = tc.alloc_tile_pool(name="ppsum", bufs=4, space="PSUM")
        xp = tc.alloc_tile_pool(name="xp", bufs=2)
        stp = tc.alloc_tile_pool(name="stp", bufs=2, space="PSUM")
        ep = tc.alloc_tile_pool(name="ep", bufs=41)
        sgp = tc.alloc_tile_pool(name="sgp", bufs=2)
        zbp = tc.alloc_tile_pool(name="zbp", bufs=3)
        zsp = tc.alloc_tile_pool(name="zsp", bufs=1)
        dpool = tc.alloc_tile_pool(name="dpool", bufs=1, space="DRAM")
        wqk = tc.alloc_tile_pool(name="wqk", bufs=1)
        zdram = dpool.tile([H, S], F32, name="zdram", tag="zdram")

        # HAM warm-up: tiny matmuls with no DMA deps keep the PE busy from
        # t=0 through the input-DMA lead-in so HAM un-throttles early.
        wups = ppsum.tile([16, P], F32, name="wups", tag="acc")
        for _ in range(20):
            nc.tensor.matmul(wups[:], wu[:, :16], wu[:])

        _padn = [0]

        def emit_pad(n):
            # HAM keep-alive filler: runs only when the FIFO is stalled on
            # DMA, preventing a mid-leadin re-throttle to 1.2 GHz.
            _padn[0] += 1
            pt = ppsum.tile([16, 64], F32, name=f"pad{_padn[0]}", tag="acc")
            for _ in range(n):
                nc.tensor.matmul(pt[:], wu[:, :16], wu[:, :64])

        # resident fp8 activations + weights for the q/k projections, fp16
        # weights for the v projection (all released together after v-proj)
        x8 = {
            "q": wqk.tile([P, NT, S], F8, name="xq8", tag="xq8"),
            "k": wqk.tile([P, NT, S], F8, name="xk8", tag="xk8"),
        }
        # weights laid out [p, n-tile, ksub, col]: each n-pair chunk is a
        # contiguous DMA slab, so k-n0's weights land ~10us earlier and the
        # first scores/exp fire at ~12us instead of ~20us
        w8 = {
            "q": wqk.tile([P, NT, NT, P], F8, name="wq8", tag="wq8"),
            "k": wqk.tile([P, NT, NT, P], F8, name="wk8", tag="wk8"),
        }
        wv = wqk.tile([P, NT, E], F16, name="wv", tag="wv")

        # DMA order matches first-use: q weights + m0 x chunks first, so the
        # first projection matmul can start ~6us in; k's m0 path next (first
        # scores ~13us); m1 chunks stream in behind.
        xsrc = {"q": xq_d, "k": xk_d}
        wsrc8 = {"q": wq8_d, "k": wk8_d}

        def emit_x8_dma(name, m):
            msl = slice(m * 512, (m + 1) * 512)
            nc.sync.dma_start(x8[name][:, :, msl], xsrc[name].ap()[:, :, msl])

        def emit_w8_dma(name, ng):
            nsl = slice(2 * ng, 2 * ng + 2)
            nc.sync.dma_start(
                w8[name][:, nsl, :, :], wsrc8[name].ap()[:, nsl, :, :]
            )

        emit_x8_dma("q", 0)
        emit_w8_dma("q", 0)
        emit_x8_dma("k", 0)
        emit_w8_dma("k", 0)
        emit_w8_dma("q", 1)
        emit_w8_dma("k", 1)
        emit_w8_dma("q", 2)
        emit_w8_dma("k", 2)
        emit_x8_dma("q", 1)
        emit_x8_dma("k", 1)

        # ---------------- scores units + pacing ----------------
        # Units become available as their QT/KT slices land; pump() always
        # emits the lowest-(t, i, j) available unit (pv consumption order).
        exps = {}
        savail = []

        def emit_s_unit():
            savail.sort(key=lambda u: (u[0], u[2], u[1]))
            t, j, i = savail.pop(0)
            jsl = slice(j * P, (j + 1) * P)
            isl = slice(i * 512, (i + 1) * 512)
            st = stp.tile([P, 1024], F32, name=f"st{t}_{j}_{i}", tag="st")
            for hh in range(2):
                base = hh * D
                nc.tensor.matmul(
                    st[:, hh * 512 : (hh + 1) * 512],
                    KT[t][base : base + D, jsl],
                    QT[t][base : base + D, isl],
                )
            ex = ep.tile([P, 1024], F16, name=f"ex{t}_{j}_{i}", tag="ex")
            nc.scalar.activation(
                ex[:], st[:], mybir.ActivationFunctionType.Exp,
                bias=zbias[:], scale=EXP_SCALE,
            )
            exps[(t, j, i)] = ex

        def pump(k):
            for _ in range(min(k, len(savail))):
                emit_s_unit()

        # ---------------- q/k projections (fp8 DoubleRow) ----------------
        def emit_proj_qk_nm(name, n, m):
            dest = QT if name == "q" else KT
            nsl = slice(n * P, (n + 1) * P)
            msl = slice(m * 512, (m + 1) * 512)
            acc = ppsum.tile([P, 512], F32, name=f"a{name}{n}_{m}", tag="acc")
            for kk in range(3):
                nc.tensor.matmul(
                    acc[:],
                    w8[name][:, n, 2 * kk : 2 * kk + 2, :],
                    x8[name][:, 2 * kk : 2 * kk + 2, msl],
                    start=(kk == 0), stop=(kk == 2),
                    perf_mode=DR,
                )
            if name == "q":
                nc.vector.tensor_scalar_add(
                    dest[n][:, msl], acc[:], bq_sb[:, n : n + 1]
                )
            else:
                nc.vector.tensor_copy(dest[n][:, msl], acc[:])

        # ---------------- v projection (fp16, x-stationary) ----------------
        def emit_xv_dma(m):
            msl = slice(m * 512, (m + 1) * 512)
            xc = xp.tile([P, NT, 512], F16, name=f"xcv{m}", tag="xc")
            nc.sync.dma_start(xc[:], xv_d.ap()[:, :, msl])
            return xc

        def emit_wv_dma():
            nc.sync.dma_start(wv[:], wv_d.ap()[:])

        def emit_v_setup():
            for g in range(MS):
                va_cols = Va[g].rearrange("p (h c) -> p h c", c=VW)
                nc.vector.memset(va_cols[:, :, D], 1.0)

        def emit_proj_v_g(xc, m, ms_i):
            g = m * 4 + ms_i
            for nch in range(2):
                ncols = 512 if nch == 0 else E - 512
                nh = ncols // D
                nsl = slice(nch * 512, nch * 512 + ncols)
                acc = ppsum.tile([P, 512], F32, name=f"av{g}_{nch}", tag="acc")
                for k in range(NT):
                    nc.tensor.matmul(
                        acc[:, :ncols],
                        xc[:, k, ms_i * P : (ms_i + 1) * P],
                        wv[:, k, nsl],
                        start=(k == 0), stop=(k == NT - 1),
                    )
                h0 = nch * 8
                dst = Va[g].rearrange("p (h c) -> p h c", c=VW)
                src = acc[:, :ncols].rearrange("p (h c) -> p h c", c=D)
                nc.vector.tensor_copy(dst[:, h0 : h0 + nh, 0:D], src[:])

        # ---------------- PV (one head pair, interleaved with pacing) ------
        # PSUM row 64 collects Z (ones column); 1/Z is computed into spare
        # PSUM row 65 so the stage copy carries it out with the O^T rows,
        # then a DRAM-bounce DMA broadcasts it across the 64 head partitions.
        def emit_pv(t, pumps, zb_pre=None):
            # Z sits in PSUM row 64 (ones column); the stage copy carries it
            # to SBUF, a DMA hop moves it to partitions 0/1, DVE computes the
            # reciprocal, and a DRAM-bounce DMA broadcasts it per head.  For
            # the last pair the whole chain is precomputed (emit_z5), so
            # zb_pre skips it.
            ci = 0
            for i in range(IC):
                dq = nc.scalar if (t >= 4 and i == 1) else nc.sync
                isl = slice(i * 512, (i + 1) * 512)
                if zb_pre is None:
                    zb = zbp.tile([P, 512], F32, name=f"zb{t}_{i}", tag="zb")
                    zt = zsp.tile([2, 512], F16, name=f"zt{t}_{i}", tag="zt")
                else:
                    zb = zb_pre[i]
                for hh in range(2):
                    h = 2 * t + hh
                    base = hh * D
                    pv = ppsum.tile([P, 512], F32, name=f"pv{h}_{i}", tag="acc")
                    for j in range(JT):
                        nc.tensor.matmul(
                            pv[0:VW, :],
                            Va[j][:, h * VW : (h + 1) * VW],
                            exps[(t, j, i)][:, hh * 512 : (hh + 1) * 512],
                            start=(j == 0), stop=(j == JT - 1),
                        )
                    stage = sgp.tile([VW, 512], F16, name=f"stg{h}_{i}", tag="stg")
                    nc.vector.tensor_copy(stage[:], pv[0:VW, :])
                    nc.sync.dma_start(OTu[t][i][base : base + D, :], stage[0:D, :])
                    if zb_pre is None:
                        dq.dma_start(zt[hh : hh + 1, :], stage[D : D + 1, :])
                    pump(pumps[ci])
                    ci += 1
                if zb_pre is None:
                    z32 = zsp.tile([2, 512], F32, name=f"z32_{t}_{i}", tag="z32")
                    rz = zsp.tile([2, 512], F32, name=f"rz{t}_{i}", tag="rz")
                    nc.vector.tensor_copy(z32[:], zt[:])
                    nc.vector.reciprocal_approx_fast(rz[:], z32[:])
                    dq.dma_start(zdram[2 * t : 2 * t + 2, isl], rz[:])
                    for hh in range(2):
                        dq.dma_start(
                            zb[hh * D : (hh + 1) * D, :],
                            zdram[2 * t + hh, isl].partition_broadcast(D),
                        )
                nc.vector.tensor_mul(OTu[t][i][:], OTu[t][i][:], zb[:])


        # ---------------- emission sequence ----------------
        # m0 pass: q/k n-tiles over sequence cols 0-511; (t, j<4, i0) score
        # units only need those cols, so exps start ~13us in.
        # all q n-tiles first: their inputs land ~8us in, and the ~5us of
        # q matmuls exactly covers the wait for the k-path DMAs (~15us),
        # keeping the PE busy/warm until scores can start
        for n in range(NT):
            emit_proj_qk_nm("q", n, 0)
        for n in range(NT):
            emit_proj_qk_nm("k", n, 0)
            savail.extend((n, j, 0) for j in range(4))
            pump(2)
        emit_pad(8)
        for n in range(NT):
            emit_proj_qk_nm("q", n, 1)
            emit_proj_qk_nm("k", n, 1)
            savail.extend((n, j, 0) for j in range(4, JT))
            savail.extend((n, j, 1) for j in range(JT))
            if n <= 1:
                emit_pad(8)
            pump(2)
        emit_v_setup()
        emit_wv_dma()
        for m in range(MC):
            xc = emit_xv_dma(m)
            for ms_i in range(4):
                emit_proj_v_g(xc, m, ms_i)
                pump(2 if m == 0 else 3)
        wqk.release()
        for t in range(NT):
            nc.sync.dma_start(woT[t][:], woT_d.ap()[t * P : (t + 1) * P, :])
        nc.sync.dma_start(ob_sb[:], ob_d.ap().partition_broadcast(P))

        pv_pumps = {0: (5, 5, 5, 5), 1: (4, 4, 4, 4), 2: (4, 4, 4, 4),
                    3: (2, 2, 2, 2), 4: (0, 0, 0, 0), 5: (0, 0, 0, 0)}
        for t in range(NT):
            emit_pv(t, pv_pumps[t])
            if t == 3:
                pump(len(savail))
        assert not savail
        # keep the PE's HAM activity monitor warm through the ~10us z-chain
        # wait between pv5 and the output projection, so the out matmuls
        # start at 2.4 GHz instead of re-throttled 1.2 GHz
        emit_pad(64)

        # ---------------- Phase O: output projection ----------------
        # Runs out of the still-live stp (PSUM accs) and ep (fp32 staging)
        # pools -- no pool-release fence between pv5 and the first out matmul.
        for m in range(MS):
            mi, mo = m // 4, m % 4
            acc = stp.tile([P, S], F32, name=f"oacc{m}", tag="st")
            # e=5 sits in its own accumulation group so the wait on the last
            # head pair's normalize chain lands on that matmul alone, not on
            # the group head (m0 e0 can start the moment pv5's matmuls end)
            for e in range(NT):
                for nch in range(2):
                    ncols = 512 if nch == 0 else E - 512
                    nsl = slice(nch * 512, nch * 512 + ncols)
                    nc.tensor.matmul(
                        acc[:, nsl],
                        OTu[e][mi][:, mo * P : (mo + 1) * P],
                        woT[e][:, nsl],
                        start=(e == 0),
                        stop=(e >= NT - 2),
                        skip_group_check=True,
                    )
            fin0 = ep.tile([P, 512], F16, name=f"fin{m}a", tag="ex")
            fin1 = ep.tile([P, E - 512], F16, name=f"fin{m}b", tag="ex")
            nc.vector.tensor_add(fin0[:], acc[:, 0:512], ob_sb[:, 0:512])
            nc.vector.tensor_add(fin1[:], acc[:, 512:E], ob_sb[:, 512:E])
            nc.sync.dma_start(out_d.ap()[m * P : (m + 1) * P, 0:512], fin0[:])
            nc.sync.dma_start(out_d.ap()[m * P : (m + 1) * P, 512:E], fin1[:])

        dpool.release()
        zsp.release()
        zbp.release()
        sgp.release()
        ep.release()
        stp.release()
        xp.release()
        ppsum.release()

    nc.compile()
    return nc


def _prep_inputs(q, k, v, in_proj_weight, in_proj_bias, out_w, out_b, lora_a, lora_b):
    q = np.asarray(q, np.float32)
    k = np.asarray(k, np.float32)
    v = np.asarray(v, np.float32)
    in_proj_weight = np.asarray(in_proj_weight, np.float32)
    in_proj_bias = np.asarray(in_proj_bias, np.float32)
    out_w = np.asarray(out_w, np.float32)
    out_b = np.asarray(out_b, np.float32)
    lora_a = np.asarray(lora_a, np.float32)
    lora_b = np.asarray(lora_b, np.float32)

    # Fold LoRA into the projection weights; drop the K bias
    # (softmax-invariant); fold the V bias into the output-projection bias
    # (attention rows sum to 1).  Q/K weights scaled by WS for fp8; the
    # compensation (and 1/sqrt(D)) is applied by the exp activation's scale.
    w_eff = in_proj_weight + lora_b @ lora_a  # [3E, E]
    wT = w_eff.T  # [E, 3E]
    w8 = np.clip(WS * wT[:, : 2 * E], -240, 240).astype(ml_dtypes.float8_e4m3)
    bq = (WS * in_proj_bias[:E]).reshape(NT, P).T  # [P, NT]
    bv = in_proj_bias[2 * E :]
    ob_eff = out_b + out_w @ bv

    f8c = lambda a: np.clip(a, -240, 240).astype(ml_dtypes.float8_e4m3)

    def pmaj(a):  # [E, X] -> [P, NT, X] partition-major (contiguous DMA lines)
        return np.ascontiguousarray(a.reshape(NT, P, -1).transpose(1, 0, 2))

    def wmaj(a):  # [E_in, E_out] -> [P, n, ksub, 128] with contiguous n-chunks
        return np.ascontiguousarray(
            a.reshape(NT, P, NT, P).transpose(1, 2, 0, 3)
        )

    shared = {
        "wq8T": wmaj(w8[:, :E]),
        "wk8T": wmaj(w8[:, E : 2 * E]),
        "wvT": pmaj(wT[:, 2 * E :].astype(np.float16)),
        "woT": np.ascontiguousarray(out_w.T, np.float16),
        "bq": np.ascontiguousarray(bq, np.float32),
        "ob": np.ascontiguousarray(ob_eff, np.float16),
    }
    in_maps = []
    for b in range(8):
        m = dict(shared)
        m["xqT"] = pmaj(f8c(q[b].T))
        m["xkT"] = pmaj(f8c(k[b].T))
        m["xvT"] = pmaj(v[b].T.astype(np.float16))
        in_maps.append(m)
    return in_maps


_NC_CACHE = {}


def run(inputs, trace=False, **spmd_kwargs):
    if "nc" not in _NC_CACHE:
        _NC_CACHE["nc"] = build_nc()
    nc = _NC_CACHE["nc"]
    in_maps = _prep_inputs(
        inputs["q"],
        inputs["k"],
        inputs["v"],
        inputs["in_proj_weight"],
        inputs["in_proj_bias"],
        inputs["out_w"],
        inputs["out_b"],
        inputs["lora_a"],
        inputs["lora_b"],
    )
    res = run_bass_kernel_spmd(
        nc, in_maps, core_ids=list(range(8)), trace=trace, **spmd_kwargs
    )
    out = np.stack([res.results[b]["out"] for b in range(8)]).astype(np.float32)
    return out, res


def kernel(
    q,
    k,
    v,
    in_proj_weight,
    in_proj_bias,
    out_w,
    out_b,
    lora_a,
    lora_b,
    num_heads=12,
    **_unused,
):
    assert int(num_heads) == H
    out, _ = run(
        {
            "q": q,
            "k": k,
            "v": v,
            "in_proj_weight": in_proj_weight,
            "in_proj_bias": in_proj_bias,
            "out_w": out_w,
            "out_b": out_b,
            "lora_a": lora_a,
            "lora_b": lora_b,
        }
    )
    return out
